# revision 4
# baseline (speedup 1.0000x reference)
"""Trainium2 Bass kernel for y = 2*(einsum('bct,oc->bot', pre, W_pre) + b_pre).

Shapes (hardcoded): pre [16, 512, 4096] f32, W_pre [512, 512] f32, b_pre [512] f32.
Sharding: data-parallel over B across 8 cores (2 batches per core).

Per core: out[b, o, t] = 2*(sum_c W[o,c]*pre[b,c,t] + bias[o]) for 2 batches.
PE matmul computes lhsT.T @ rhs with lhsT = W.T tiles [K=128, M=128] and
rhs = pre tiles [K=128, N=512]; accumulate 4 K-tiles into one PSUM bank,
then ScalarE applies out = 2*psum + 2*bias on eviction PSUM->SBUF.
"""

import os
import sys

for _p in ("/opt/trn_rl_repo", "/root/.axon_site/_ro/trn_rl_repo"):
    if os.path.isdir(_p) and _p not in sys.path:
        sys.path.append(_p)

from contextlib import ExitStack

import numpy as np

import concourse.bass as bass
import concourse.tile as tile
from concourse import bacc, mybir
from concourse.bass_utils import run_bass_kernel_spmd

B, C, T = 16, 512, 4096  # batch, channels (in == out), sequence
NCORES = 8
BPC = B // NCORES  # batches per core
P = 128
KT = C // P  # contraction tiles
MT = C // P  # output-channel tiles
NCHUNK = 512  # matmul moving-operand free dim (max for 4-byte dtypes)
NCH = T // NCHUNK

# float32: exact, 4 cycles/row on PE. float32r (tf32): 1 cycle/row at N>=256.
MM_DTYPE = mybir.dt.float32

LAST_RESULT = None  # BassKernelResults of the most recent run (for test harness)
_cache = {}


def _build(mm_dtype):
    # Bacc (not plain Bass): its finalize() runs move_matmul_waits_to_ldweights +
    # generate_event_semaphores, which walrus needs — an fp32 self-loading
    # matmul's implicit LDWEIGHTS tolerates only one semaphore wait.
    nc = bacc.Bacc("TRN2", target_bir_lowering=False, debug=False, num_devices=NCORES)
    pre = nc.dram_tensor("pre", [BPC, C, T], mybir.dt.float32, kind="ExternalInput").ap()
    wt = nc.dram_tensor("wt", [C, C], mybir.dt.float32, kind="ExternalInput").ap()
    b2 = nc.dram_tensor("b2", [P, MT], mybir.dt.float32, kind="ExternalInput").ap()
    out = nc.dram_tensor("out", [BPC, C, T], mybir.dt.float32, kind="ExternalOutput").ap()

    with ExitStack() as ctx:
        tc = ctx.enter_context(tile.TileContext(nc))
        wpool = ctx.enter_context(tc.tile_pool(name="w", bufs=1))
        bpool = ctx.enter_context(tc.tile_pool(name="bias", bufs=1))
        xpool = ctx.enter_context(tc.tile_pool(name="x", bufs=2 * KT))
        opool = ctx.enter_context(tc.tile_pool(name="o", bufs=8))
        pspool = ctx.enter_context(tc.tile_pool(name="ps", bufs=8, space="PSUM"))

        # W.T resident in SBUF: 4 tiles [128, 512]; lhsT for (kt, mt) is
        # wtiles[kt][:, mt*128:(mt+1)*128]
        wtiles = []
        for kt in range(KT):
            w = wpool.tile([P, C], mybir.dt.float32, tag=f"w{kt}")
            nc.sync.dma_start(w[:], wt[kt * P : (kt + 1) * P, :])
            wtiles.append(w)

        btile = bpool.tile([P, MT], mybir.dt.float32)
        nc.sync.dma_start(btile[:], b2[:])

        for b in range(BPC):
            xtiles = []
            for kt in range(KT):
                x = xpool.tile([P, T], mybir.dt.float32, tag="x")
                nc.sync.dma_start(x[:], pre[b, kt * P : (kt + 1) * P, :])
                xtiles.append(x)
            for nch in range(NCH):
                for mt in range(MT):
                    ps = pspool.tile([P, NCHUNK], mybir.dt.float32, tag="ps")
                    for kt in range(KT):
                        lhsT = wtiles[kt][:, mt * P : (mt + 1) * P]
                        rhs = xtiles[kt][:, bass.ts(nch, NCHUNK)]
                        if mm_dtype != mybir.dt.float32:
                            lhsT = lhsT.bitcast(mm_dtype)
                            rhs = rhs.bitcast(mm_dtype)
                        nc.tensor.matmul(
                            ps[:], lhsT, rhs, start=(kt == 0), stop=(kt == KT - 1)
                        )
                    ot = opool.tile([P, NCHUNK], mybir.dt.float32, tag="o")
                    nc.scalar.activation(
                        ot[:],
                        ps[:],
                        mybir.ActivationFunctionType.Identity,
                        bias=btile[:, mt : mt + 1],
                        scale=2.0,
                    )
                    nc.sync.dma_start(
                        out[b, mt * P : (mt + 1) * P, bass.ts(nch, NCHUNK)], ot[:]
                    )
    # The axon/PJRT exec path serializes nc as-is; finalize here so Bacc's
    # compile passes (register alloc, event-semaphore wait splitting) run.
    nc.finalize()
    return nc


def kernel(pre, W_pre, b_pre):
    global LAST_RESULT
    pre = np.ascontiguousarray(pre, dtype=np.float32)
    wT = np.ascontiguousarray(np.asarray(W_pre, dtype=np.float32).T)
    b2 = np.ascontiguousarray(
        (2.0 * np.asarray(b_pre, dtype=np.float32)).reshape(MT, P).T
    )
    key = str(MM_DTYPE)
    if key not in _cache:
        _cache[key] = _build(MM_DTYPE)
    nc = _cache[key]
    in_maps = [
        {"pre": pre[i * BPC : (i + 1) * BPC], "wt": wT, "b2": b2}
        for i in range(NCORES)
    ]
    res = run_bass_kernel_spmd(nc, in_maps, list(range(NCORES)))
    LAST_RESULT = res
    return np.ascontiguousarray(
        np.concatenate([res.results[i]["out"] for i in range(NCORES)], axis=0),
        dtype=np.float32,
    )


# revision 10
# speedup vs baseline: 2.2270x; 2.2270x over previous
"""Trainium2 Bass kernel for y = 2*(einsum('bct,oc->bot', pre, W_pre) + b_pre).

Shapes (hardcoded): pre [16, 512, 4096] f32, W_pre [512, 512] f32, b_pre [512] f32.
Sharding: data-parallel over B across 8 cores (2 batches per core).

Per core: out[b, o, t] = 2*(sum_c W[o,c]*pre[b,c,t] + bias[o]) for 2 batches.
PE matmul computes lhsT.T @ rhs with lhsT = W.T tiles [K=128, M=128] and
rhs = pre tiles [K=128, N=512]; accumulate 4 K-tiles into one PSUM bank,
then ScalarE applies out = 2*psum + 2*bias on eviction PSUM->SBUF.
"""

import os
import sys

for _p in ("/opt/trn_rl_repo", "/root/.axon_site/_ro/trn_rl_repo"):
    if os.path.isdir(_p) and _p not in sys.path:
        sys.path.append(_p)

from contextlib import ExitStack

import numpy as np

import concourse.bass as bass
import concourse.tile as tile
from concourse import bacc, mybir
from concourse.bass_utils import run_bass_kernel_spmd

B, C, T = 16, 512, 4096  # batch, channels (in == out), sequence
NCORES = 8
BPC = B // NCORES  # batches per core
P = 128
KT = C // P  # contraction tiles
MT = C // P  # output-channel tiles
NCHUNK = 512  # matmul moving-operand free dim (max for 4-byte dtypes)
NCH = T // NCHUNK

# float32: exact, 4 cycles/row on PE. float32r (tf32): 1 cycle/row at N>=256.
MM_DTYPE = mybir.dt.float32r

LAST_RESULT = None  # BassKernelResults of the most recent run (for test harness)
_cache = {}


def _build(mm_dtype):
    # Bacc (not plain Bass): its finalize() runs move_matmul_waits_to_ldweights +
    # generate_event_semaphores, which walrus needs — an fp32 self-loading
    # matmul's implicit LDWEIGHTS tolerates only one semaphore wait.
    nc = bacc.Bacc("TRN2", target_bir_lowering=False, debug=False, num_devices=NCORES)
    # When running tf32 matmuls, the BIR verifier requires matmul inputs to be
    # produced as float32r; declaring the DRAM side as float32r (with the host
    # pre-rounding the payload to tf32) satisfies it without a device-side pass.
    in_dt = mm_dtype if mm_dtype == mybir.dt.float32r else mybir.dt.float32
    pre = nc.dram_tensor("pre", [BPC, C, T], in_dt, kind="ExternalInput").ap()
    wt = nc.dram_tensor("wt", [C, C], in_dt, kind="ExternalInput").ap()
    b2 = nc.dram_tensor("b2", [P, MT], mybir.dt.float32, kind="ExternalInput").ap()
    out = nc.dram_tensor("out", [BPC, C, T], mybir.dt.float32, kind="ExternalOutput").ap()

    with ExitStack() as ctx:
        tc = ctx.enter_context(tile.TileContext(nc))
        wpool = ctx.enter_context(tc.tile_pool(name="w", bufs=1))
        bpool = ctx.enter_context(tc.tile_pool(name="bias", bufs=1))
        xpool = ctx.enter_context(tc.tile_pool(name="x", bufs=2 * KT))
        opool = ctx.enter_context(tc.tile_pool(name="o", bufs=8))
        pspool = ctx.enter_context(tc.tile_pool(name="ps", bufs=8, space="PSUM"))

        # W.T resident in SBUF: 4 tiles [128, 512]; lhsT for (kt, mt) is
        # wtiles[kt][:, mt*128:(mt+1)*128]
        wtiles = []
        for kt in range(KT):
            w = wpool.tile([P, C], in_dt, tag=f"w{kt}")
            nc.sync.dma_start(w[:], wt[kt * P : (kt + 1) * P, :])
            wtiles.append(w)

        btile = bpool.tile([P, MT], mybir.dt.float32)
        nc.sync.dma_start(btile[:], b2[:])

        for b in range(BPC):
            xtiles = []
            for kt in range(KT):
                x = xpool.tile([P, T], in_dt, tag="x")
                nc.sync.dma_start(x[:], pre[b, kt * P : (kt + 1) * P, :])
                xtiles.append(x)
            for nch in range(NCH):
                for mt in range(MT):
                    ps = pspool.tile([P, NCHUNK], mybir.dt.float32, tag="ps")
                    for kt in range(KT):
                        lhsT = wtiles[kt][:, mt * P : (mt + 1) * P]
                        rhs = xtiles[kt][:, bass.ts(nch, NCHUNK)]
                        if mm_dtype != in_dt:
                            lhsT = lhsT.bitcast(mm_dtype)
                            rhs = rhs.bitcast(mm_dtype)
                        nc.tensor.matmul(
                            ps[:], lhsT, rhs, start=(kt == 0), stop=(kt == KT - 1)
                        )
                    ot = opool.tile([P, NCHUNK], mybir.dt.float32, tag="o")
                    nc.scalar.activation(
                        ot[:],
                        ps[:],
                        mybir.ActivationFunctionType.Identity,
                        bias=btile[:, mt : mt + 1],
                        scale=2.0,
                    )
                    nc.sync.dma_start(
                        out[b, mt * P : (mt + 1) * P, bass.ts(nch, NCHUNK)], ot[:]
                    )
    # The axon/PJRT exec path serializes nc as-is; finalize here so Bacc's
    # compile passes (register alloc, event-semaphore wait splitting) run.
    nc.finalize()
    return nc


def _round_tf32(a):
    """Round fp32 array to tf32 (10-bit mantissa), round-to-nearest-even."""
    u = a.view(np.uint32)
    r = u + (0xFFF + ((u >> 13) & 1))
    r &= np.uint32(0xFFFFE000)
    # NaN/Inf payloads must not be touched by the carry into the exponent
    special = (u & np.uint32(0x7F800000)) == np.uint32(0x7F800000)
    r[special] = u[special] & np.uint32(0xFFFFE000)
    return r.view(np.float32)


def kernel(pre, W_pre, b_pre):
    global LAST_RESULT
    pre = np.ascontiguousarray(pre, dtype=np.float32)
    wT = np.ascontiguousarray(np.asarray(W_pre, dtype=np.float32).T)
    if MM_DTYPE == mybir.dt.float32r:
        pre = _round_tf32(pre)
        wT = _round_tf32(wT)
    b2 = np.ascontiguousarray(
        (2.0 * np.asarray(b_pre, dtype=np.float32)).reshape(MT, P).T
    )
    key = str(MM_DTYPE)
    if key not in _cache:
        _cache[key] = _build(MM_DTYPE)
    nc = _cache[key]
    in_maps = [
        {"pre": pre[i * BPC : (i + 1) * BPC], "wt": wT, "b2": b2}
        for i in range(NCORES)
    ]
    res = run_bass_kernel_spmd(nc, in_maps, list(range(NCORES)))
    LAST_RESULT = res
    return np.ascontiguousarray(
        np.concatenate([res.results[i]["out"] for i in range(NCORES)], axis=0),
        dtype=np.float32,
    )
